# revision 42
# baseline (speedup 1.0000x reference)
"""Trainium2 Bass kernel for nn_DiscreteSelectorTransform (MoE-style routed MAF).

Math (reference):
    h   = einsum('bd,kde->bke', y, W*mask) + b      # all K experts, dense
    sel = h[arange(B), x]                           # pick expert by label
    shift, log_scale = split(sel, 2, -1)
    u   = y * exp(log_scale) + shift
    out = where(x == 0, y, u)                       # expert 0 == identity

Instead of the dense all-experts compute (8x waste), we route on the host:
 - rows with label 0 are copied through on the host (identity expert),
 - remaining rows are grouped by expert and distributed over the 8 cores,
   one expert per core (big experts split across spare cores), so each
   core loads exactly one masked weight matrix (2MB) and runs a plain
   [rows, 512] @ [512, 1024] matmul + exp/mul/add epilogue for its rows.

Device layout: transposed orientation.  Each core receives y^T [512, R]
(features on partitions), computes h^T = Wm^T @ y^T into PSUM as 8
[128, N] e-chunks (W d-chunk stationary, y^T moving, float32r full-rate
mode), then the epilogue
    u^T = y^T * exp(ls^T + b_ls) + (sh^T + b_sh)
runs on ACT (exp / bias-add, per-partition bias) + DVE (mul / add) in the
same transposed layout and streams u^T [512, R] back out.  The host
inverts the permutation.
"""

import os
import numpy as np

_TRN_REPO = "/opt/trn_rl_repo"


def _ensure_path():
    try:
        import concourse  # noqa: F401
    except ImportError:
        import sys

        sys.path.insert(0, _TRN_REPO)


_PROGRAM_CACHE = {}

LAST_EXEC_NS = None
LAST_RESULTS = None


def _build_program(R):
    """One SPMD program: R rows (multiple of 256), one expert weight per core.

    Rows are processed in groups of 512 (plus one 256 tail when R % 512):
    512-wide DMA lines are at the HWDGE efficiency knee, and both widths
    keep float32r matmuls at full rate (moving dim >= 256).

    The weight arrives column-PACKED per d-chunk: chunk c keeps only its
    2*(c+1) nonzero e-blocks [shift j<=c | log_scale j<=c], so the DMA
    moves 20/32 of the dense masked weight.
    """
    _ensure_path()
    import concourse.mybir as mybir
    import concourse.tile as tile
    from concourse import bacc
    from concourse.vector_clock import ScopedClock

    class _LeanTileContext(tile.TileContext):
        """Tile's stock exit costs ~12us on HW: a drain plus two all-engine
        EVSEM butterflies around the gpsimd sem cleanup.  Both barriers are
        redundant here: the SP drain's global-clock waits already order the
        cleanup after every engine's last tile-sem op, and the bass/runtime
        end-of-module barrier (which every engine executes right after this
        block) provides the all-engine rendezvous that protects the next
        execution.  So: drain -> one-sem handoff to gpsimd -> cleanup."""

        def _drain_and_barrier(self, tick_clock, wait_clock):
            nc = self.nc
            drain_inst = nc.sync.drain()
            wait_clock.add_sem_waits(
                drain_inst.ins, ScopedClock({None: tick_clock.global_clock})
            )
            handoff = nc.alloc_semaphore("lean_exit_handoff")
            drain_inst.then_inc(handoff, 1)
            nc.gpsimd.wait_ge(handoff, 1)
            popped = nc._tile_sem_poison_stack.pop()
            assert popped is self._sem_poison
            nc.clear_and_free_semaphores(
                list(self.sems.allocated().values()) + [handoff]
            )

    assert R % 256 == 0
    widths = [512] * (R // 512) + ([256] if R % 512 else [])
    D = 512
    E = 1024

    nc = bacc.Bacc(
        "TRN2",
        target_bir_lowering=False,
        debug=False,
        enable_asserts=False,
        num_devices=8,
        enable_partition_id=False,
    )
    f32 = mybir.dt.float32
    f32r = mybir.dt.float32r
    Exp = mybir.ActivationFunctionType.Exp

    # y / W live as float32r end-to-end: same fp32 bits host-side, but the
    # BIR verifier requires f32r matmult operands to be produced as f32r.
    yT = nc.dram_tensor("yT", [D, R], f32r, kind="ExternalInput").ap()
    Wm = nc.dram_tensor("Wm", [D, E], f32r, kind="ExternalInput").ap()
    bT = nc.dram_tensor("bT", [128, 8], f32, kind="ExternalInput").ap()
    uT = nc.dram_tensor("uT", [D, R], f32, kind="ExternalOutput").ap()

    with _LeanTileContext(nc) as tc:
        with (
            tc.tile_pool(name="wp", bufs=1) as wp,
            tc.tile_pool(name="yp", bufs=6) as yp,
            tc.tile_pool(name="up", bufs=3) as up,
            tc.tile_pool(name="ep", bufs=2) as ep,
            tc.tile_pool(name="pp", bufs=1, space="PSUM") as pp,
        ):
            # One-time loads: packed masked weight (4 d-chunks) + bias.
            # The first matmul needs W chunk 0 + y chunk 0: put W chunk 0
            # first on the SP queue (ahead of y), the rest on the ACT HWDGE
            # (whose queue opens later, after its activation-table load).
            wts = []
            for c in range(4):
                ncols = 256 * (c + 1)  # 2*(c+1) nonzero e-blocks
                wt = wp.tile([128, ncols], f32r, name=f"w{c}", tag=f"w{c}")
                eng = nc.sync if c == 0 else nc.scalar
                eng.dma_start(out=wt, in_=Wm[128 * c : 128 * (c + 1), :ncols])
                wts.append(wt)
            bt = wp.tile([128, 8], f32, name="bt", tag="bt")
            nc.scalar.dma_start(out=bt, in_=bT)

            off = 0
            for g, N in enumerate(widths):
                sl = slice(off, off + N)
                off += N
                # One DMA per group: d-chunk c lands at columns [c*N, (c+1)*N).
                # The pipeline-fill groups are split per chunk instead, so the
                # first matmul only waits for 256KB, not the whole group.
                yc = yp.tile([128, 4 * N], f32r, name="yc", tag="yc")
                if g < 2:
                    for c in range(4):
                        nc.sync.dma_start(
                            out=yc[:, c * N : (c + 1) * N],
                            in_=yT[128 * c : 128 * (c + 1), sl],
                        )
                else:
                    nc.sync.dma_start(
                        out=yc, in_=yT[:, sl].rearrange("(c q) n -> q c n", c=4)
                    )
                yts = [yc[:, c * N : (c + 1) * N] for c in range(4)]

                # e-chunk j of h^T accumulates over d-chunks c >= j only:
                # the AR mask m[d, i] = (i < d) zeroes every (c, j) weight
                # block with c < j, in both the shift and log_scale halves.
                # In chunk c's packed weight, shift block j sits at column
                # 128*j and log_scale block j at 128*(c+1+j).
                # Feature chunks are processed in PAIRS (2 PSUM banks per
                # tile) so each epilogue op covers 2*N elements — halves the
                # op count and, more importantly, the semaphore traffic.
                # b_shift is added host-side during unshard.
                for p in range(2):
                    psh = pp.tile([128, 2 * N], f32, name=f"psh{p}", tag=f"psh{p}")
                    pls = pp.tile([128, 2 * N], f32, name=f"pls{p}", tag=f"pls{p}")
                    et = ep.tile([128, 2 * N], f32, name=f"et{p}", tag=f"e{p}")
                    for h in range(2):
                        j = 2 * p + h
                        hs = slice(h * N, (h + 1) * N)
                        for c in range(j, 4):
                            nc.tensor.matmul(
                                psh[:, hs],
                                wts[c][:, 128 * j : 128 * (j + 1)],
                                yts[c],
                                start=(c == j),
                                stop=(c == 3),
                            )
                        for c in range(j, 4):
                            blk = c + 1 + j
                            nc.tensor.matmul(
                                pls[:, hs],
                                wts[c][:, 128 * blk : 128 * (blk + 1)],
                                yts[c],
                                start=(c == j),
                                stop=(c == 3),
                            )
                        nc.scalar.activation(
                            et[:, hs], pls[:, hs], Exp, bias=bt[:, 4 + j : 5 + j]
                        )
                    ut = up.tile([128, 2 * N], f32, name=f"ut{p}", tag=f"u{p}")
                    nc.vector.tensor_mul(
                        ut, yc[:, 2 * p * N : (2 * p + 2) * N].bitcast(f32), et
                    )
                    nc.vector.tensor_add(ut, ut, psh)
                    # alternate output DMAs between the two HWDGE queues
                    dma_eng = nc.sync if p == 0 else nc.scalar
                    dma_eng.dma_start(
                        out=uT[256 * p : 256 * (p + 1), sl].rearrange(
                            "(a q) n -> q a n", a=2
                        ),
                        in_=ut,
                    )

    nc.compile()
    return nc


def _get_program(R):
    if R not in _PROGRAM_CACHE:
        _PROGRAM_CACHE[R] = _build_program(R)
    return _PROGRAM_CACHE[R]


def _pack_weight(Wmk):
    """Column-pack the masked weight: d-chunk c keeps its 2*(c+1) nonzero
    e-blocks as [shift j=0..c | log_scale j=0..c]."""
    Wp = np.zeros((512, 1024), dtype=np.float32)
    for c in range(4):
        rows = slice(128 * c, 128 * (c + 1))
        for j in range(c + 1):
            Wp[rows, 128 * j : 128 * (j + 1)] = Wmk[rows, 128 * j : 128 * (j + 1)]
            Wp[rows, 128 * (c + 1 + j) : 128 * (c + 2 + j)] = Wmk[
                rows, 512 + 128 * j : 512 + 128 * (j + 1)
            ]
    return Wp


def _route(counts, n_cores):
    """Cores per expert: start 1 each (nonzero), grow the most loaded."""
    m = np.where(counts > 0, 1, 0).astype(np.int64)
    if m.sum() == 0:
        return m
    while m.sum() < n_cores:
        load = np.where(m > 0, counts / np.maximum(m, 1), -1.0)
        m[int(np.argmax(load))] += 1
    return m


def kernel(y, W, b, x):
    global LAST_EXEC_NS, LAST_RESULTS
    y = np.ascontiguousarray(np.asarray(y, dtype=np.float32))
    W = np.asarray(W, dtype=np.float32)
    b = np.asarray(b, dtype=np.float32)
    xl = np.asarray(x).astype(np.int64).ravel()

    B, D = y.shape
    K, _, E = W.shape
    n_cores = 8

    out = np.empty((B, D), dtype=np.float32)
    id_rows = xl == 0
    out[id_rows] = y[id_rows]

    rows_per_expert = [np.nonzero(xl == k)[0] for k in range(1, K)]
    counts = np.array([len(r) for r in rows_per_expert], dtype=np.int64)
    if counts.sum() == 0:
        return out

    m = _route(counts, n_cores)
    assignments = []  # (expert index 1..K-1, row ids)
    for ki, rows in enumerate(rows_per_expert):
        if m[ki] == 0:
            continue
        for s in np.array_split(rows, m[ki]):
            assignments.append((ki + 1, s))
    while len(assignments) < n_cores:
        assignments.append((1, np.array([], dtype=np.int64)))

    max_rows = max(len(s) for _, s in assignments)
    R = max(256, -(-max_rows // 256) * 256)

    mask = np.tril(np.ones((D, D), dtype=np.float32), -1)
    maskc = np.concatenate([mask, mask], axis=1)

    in_maps = []
    for ke, rows in assignments:
        yTc = np.zeros((D, R), dtype=np.float32)
        if len(rows):
            yTc[:, : len(rows)] = y[rows].T
        in_maps.append(
            {
                "yT": yTc,
                "Wm": _pack_weight(W[ke] * maskc),
                "bT": np.ascontiguousarray(b[ke].reshape(8, 128).T),
            }
        )

    nc = _get_program(R)

    if os.environ.get("BASS_KERNEL_SIM") == "1":
        results = _run_sim(nc, in_maps)
    else:
        results = _run_hw(nc, in_maps)
    LAST_RESULTS = results

    for (ke, rows), res in zip(assignments, results):
        if len(rows):
            # device returns y*exp(ls + b_ls) + shift; add b_shift here
            out[rows] = res["uT"][:, : len(rows)].T + b[ke, :D][None, :]
    return out


def _install_ntff_hook_shim():
    """This image's antenv lacks axon_hooks; wire the ctypes NTFF hook up
    ourselves so run_bass_kernel_spmd(trace=True) can profile."""
    import sys
    import types

    if "antenv.axon_hooks" in sys.modules:
        return
    try:
        from trn_agent_boot.trn_boot import _ntff_profile_via_ctypes

        hook = _ntff_profile_via_ctypes("/opt/axon/libaxon_pjrt.so")
    except Exception:
        hook = None
    mod = types.ModuleType("antenv.axon_hooks")
    mod.get_axon_ntff_profile_hook = lambda: hook
    sys.modules["antenv.axon_hooks"] = mod


def _run_hw(nc, in_maps):
    global LAST_EXEC_NS
    _ensure_path()
    from concourse import bass_utils

    trace = os.environ.get("BASS_KERNEL_TRACE") == "1"
    if trace:
        _install_ntff_hook_shim()
    res = bass_utils.run_bass_kernel_spmd(
        nc,
        in_maps,
        core_ids=list(range(len(in_maps))),
        trace=trace,
    )
    LAST_EXEC_NS = res.exec_time_ns
    return res.results


def _run_sim(nc, in_maps):
    """CoreSim the program (core 0's data on every core) — debug only."""
    _ensure_path()
    from concourse.bass_interp import CoreSim

    results = []
    for im in in_maps:
        sim = CoreSim(nc, trace=False, require_finite=True, require_nnan=True)
        for name, val in im.items():
            sim.tensor(name)[:] = val
        sim.simulate(check_with_hw=False)
        results.append({"uT": sim.tensor("uT").copy()})
    return results


# revision 44
# speedup vs baseline: 1.0421x; 1.0421x over previous
"""Trainium2 Bass kernel for nn_DiscreteSelectorTransform (MoE-style routed MAF).

Math (reference):
    h   = einsum('bd,kde->bke', y, W*mask) + b      # all K experts, dense
    sel = h[arange(B), x]                           # pick expert by label
    shift, log_scale = split(sel, 2, -1)
    u   = y * exp(log_scale) + shift
    out = where(x == 0, y, u)                       # expert 0 == identity

Instead of the dense all-experts compute (8x waste), we route on the host:
 - rows with label 0 are copied through on the host (identity expert),
 - remaining rows are grouped by expert and distributed over the 8 cores,
   one expert per core (big experts split across spare cores), so each
   core loads exactly one masked weight matrix (2MB) and runs a plain
   [rows, 512] @ [512, 1024] matmul + exp/mul/add epilogue for its rows.

Device layout: transposed orientation.  Each core receives y^T [512, R]
(features on partitions), computes h^T = Wm^T @ y^T into PSUM as 8
[128, N] e-chunks (W d-chunk stationary, y^T moving, float32r full-rate
mode), then the epilogue
    u^T = y^T * exp(ls^T + b_ls) + (sh^T + b_sh)
runs on ACT (exp / bias-add, per-partition bias) + DVE (mul / add) in the
same transposed layout and streams u^T [512, R] back out.  The host
inverts the permutation.
"""

import os
import numpy as np

_TRN_REPO = "/opt/trn_rl_repo"


def _ensure_path():
    try:
        import concourse  # noqa: F401
    except ImportError:
        import sys

        sys.path.insert(0, _TRN_REPO)


_PROGRAM_CACHE = {}

LAST_EXEC_NS = None
LAST_RESULTS = None


def _build_program(R):
    """One SPMD program: R rows (multiple of 256), one expert weight per core.

    Rows are processed in groups of 512 (plus one 256 tail when R % 512):
    512-wide DMA lines are at the HWDGE efficiency knee, and both widths
    keep float32r matmuls at full rate (moving dim >= 256).

    The weight arrives column-PACKED per d-chunk: chunk c keeps only its
    2*(c+1) nonzero e-blocks [shift j<=c | log_scale j<=c], so the DMA
    moves 20/32 of the dense masked weight.
    """
    _ensure_path()
    import concourse.mybir as mybir
    import concourse.tile as tile
    from concourse import bacc
    from concourse.vector_clock import ScopedClock

    class _LeanTileContext(tile.TileContext):
        """Tile's stock exit costs ~12us on HW: drain + two all-engine EVSEM
        butterflies around the gpsimd sem cleanup.  The second butterfly only
        protects against a next execution racing the cleanup, but the
        bass/runtime end-of-module barrier (which every engine executes right
        after this block) already provides that rendezvous — so a sem-only
        barrier plus cleanup suffices.  (A leaner drain->gpsimd semaphore
        handoff instead of the barrier measured the same within noise.)"""

        def _drain_and_barrier(self, tick_clock, wait_clock):
            drain_inst = self.nc.sync.drain()
            wait_clock.add_sem_waits(
                drain_inst.ins, ScopedClock({None: tick_clock.global_clock})
            )
            self.nc.all_engine_barrier(sem_only=True)
            popped = self.nc._tile_sem_poison_stack.pop()
            assert popped is self._sem_poison
            self.nc.clear_and_free_semaphores(list(self.sems.allocated().values()))

    assert R % 256 == 0
    widths = [512] * (R // 512) + ([256] if R % 512 else [])
    D = 512
    E = 1024

    nc = bacc.Bacc(
        "TRN2",
        target_bir_lowering=False,
        debug=False,
        enable_asserts=False,
        num_devices=8,
        enable_partition_id=False,
    )
    f32 = mybir.dt.float32
    f32r = mybir.dt.float32r
    Exp = mybir.ActivationFunctionType.Exp

    # y / W live as float32r end-to-end: same fp32 bits host-side, but the
    # BIR verifier requires f32r matmult operands to be produced as f32r.
    yT = nc.dram_tensor("yT", [D, R], f32r, kind="ExternalInput").ap()
    Wm = nc.dram_tensor("Wm", [D, E], f32r, kind="ExternalInput").ap()
    bT = nc.dram_tensor("bT", [128, 8], f32, kind="ExternalInput").ap()
    uT = nc.dram_tensor("uT", [D, R], f32, kind="ExternalOutput").ap()

    with _LeanTileContext(nc) as tc:
        with (
            tc.tile_pool(name="wp", bufs=1) as wp,
            tc.tile_pool(name="yp", bufs=6) as yp,
            tc.tile_pool(name="up", bufs=3) as up,
            tc.tile_pool(name="ep", bufs=2) as ep,
            tc.tile_pool(name="pp", bufs=1, space="PSUM") as pp,
        ):
            # One-time loads: packed masked weight (4 d-chunks) + bias.
            # The first matmul needs W chunk 0 + y chunk 0: put W chunk 0
            # first on the SP queue (ahead of y), the rest on the ACT HWDGE
            # (whose queue opens later, after its activation-table load).
            wts = []
            for c in range(4):
                ncols = 256 * (c + 1)  # 2*(c+1) nonzero e-blocks
                wt = wp.tile([128, ncols], f32r, name=f"w{c}", tag=f"w{c}")
                eng = nc.sync if c == 0 else nc.scalar
                eng.dma_start(out=wt, in_=Wm[128 * c : 128 * (c + 1), :ncols])
                wts.append(wt)
            bt = wp.tile([128, 8], f32, name="bt", tag="bt")
            nc.scalar.dma_start(out=bt, in_=bT)

            off = 0
            for g, N in enumerate(widths):
                sl = slice(off, off + N)
                off += N
                # One DMA per group: d-chunk c lands at columns [c*N, (c+1)*N).
                # The pipeline-fill groups are split per chunk instead, so the
                # first matmul only waits for 256KB, not the whole group.
                yc = yp.tile([128, 4 * N], f32r, name="yc", tag="yc")
                if g < 2:
                    for c in range(4):
                        nc.sync.dma_start(
                            out=yc[:, c * N : (c + 1) * N],
                            in_=yT[128 * c : 128 * (c + 1), sl],
                        )
                else:
                    nc.sync.dma_start(
                        out=yc, in_=yT[:, sl].rearrange("(c q) n -> q c n", c=4)
                    )
                yts = [yc[:, c * N : (c + 1) * N] for c in range(4)]

                # e-chunk j of h^T accumulates over d-chunks c >= j only:
                # the AR mask m[d, i] = (i < d) zeroes every (c, j) weight
                # block with c < j, in both the shift and log_scale halves.
                # In chunk c's packed weight, shift block j sits at column
                # 128*j and log_scale block j at 128*(c+1+j).
                # Feature chunks are processed in PAIRS (2 PSUM banks per
                # tile) so each epilogue op covers 2*N elements — halves the
                # op count and, more importantly, the semaphore traffic.
                # b_shift is added host-side during unshard.
                for p in range(2):
                    psh = pp.tile([128, 2 * N], f32, name=f"psh{p}", tag=f"psh{p}")
                    pls = pp.tile([128, 2 * N], f32, name=f"pls{p}", tag=f"pls{p}")
                    et = ep.tile([128, 2 * N], f32, name=f"et{p}", tag=f"e{p}")
                    for h in range(2):
                        j = 2 * p + h
                        hs = slice(h * N, (h + 1) * N)
                        for c in range(j, 4):
                            nc.tensor.matmul(
                                psh[:, hs],
                                wts[c][:, 128 * j : 128 * (j + 1)],
                                yts[c],
                                start=(c == j),
                                stop=(c == 3),
                            )
                        for c in range(j, 4):
                            blk = c + 1 + j
                            nc.tensor.matmul(
                                pls[:, hs],
                                wts[c][:, 128 * blk : 128 * (blk + 1)],
                                yts[c],
                                start=(c == j),
                                stop=(c == 3),
                            )
                        nc.scalar.activation(
                            et[:, hs], pls[:, hs], Exp, bias=bt[:, 4 + j : 5 + j]
                        )
                    ut = up.tile([128, 2 * N], f32, name=f"ut{p}", tag=f"u{p}")
                    nc.vector.tensor_mul(
                        ut, yc[:, 2 * p * N : (2 * p + 2) * N].bitcast(f32), et
                    )
                    nc.vector.tensor_add(ut, ut, psh)
                    # alternate output DMAs between the two HWDGE queues
                    dma_eng = nc.sync if p == 0 else nc.scalar
                    dma_eng.dma_start(
                        out=uT[256 * p : 256 * (p + 1), sl].rearrange(
                            "(a q) n -> q a n", a=2
                        ),
                        in_=ut,
                    )

    nc.compile()
    return nc


def _get_program(R):
    if R not in _PROGRAM_CACHE:
        _PROGRAM_CACHE[R] = _build_program(R)
    return _PROGRAM_CACHE[R]


def _pack_weight(Wmk):
    """Column-pack the masked weight: d-chunk c keeps its 2*(c+1) nonzero
    e-blocks as [shift j=0..c | log_scale j=0..c]."""
    Wp = np.zeros((512, 1024), dtype=np.float32)
    for c in range(4):
        rows = slice(128 * c, 128 * (c + 1))
        for j in range(c + 1):
            Wp[rows, 128 * j : 128 * (j + 1)] = Wmk[rows, 128 * j : 128 * (j + 1)]
            Wp[rows, 128 * (c + 1 + j) : 128 * (c + 2 + j)] = Wmk[
                rows, 512 + 128 * j : 512 + 128 * (j + 1)
            ]
    return Wp


def _route(counts, n_cores):
    """Cores per expert: start 1 each (nonzero), grow the most loaded."""
    m = np.where(counts > 0, 1, 0).astype(np.int64)
    if m.sum() == 0:
        return m
    while m.sum() < n_cores:
        load = np.where(m > 0, counts / np.maximum(m, 1), -1.0)
        m[int(np.argmax(load))] += 1
    return m


def kernel(y, W, b, x):
    global LAST_EXEC_NS, LAST_RESULTS
    y = np.ascontiguousarray(np.asarray(y, dtype=np.float32))
    W = np.asarray(W, dtype=np.float32)
    b = np.asarray(b, dtype=np.float32)
    xl = np.asarray(x).astype(np.int64).ravel()

    B, D = y.shape
    K, _, E = W.shape
    n_cores = 8

    out = np.empty((B, D), dtype=np.float32)
    id_rows = xl == 0
    out[id_rows] = y[id_rows]

    rows_per_expert = [np.nonzero(xl == k)[0] for k in range(1, K)]
    counts = np.array([len(r) for r in rows_per_expert], dtype=np.int64)
    if counts.sum() == 0:
        return out

    m = _route(counts, n_cores)
    assignments = []  # (expert index 1..K-1, row ids)
    for ki, rows in enumerate(rows_per_expert):
        if m[ki] == 0:
            continue
        for s in np.array_split(rows, m[ki]):
            assignments.append((ki + 1, s))
    while len(assignments) < n_cores:
        assignments.append((1, np.array([], dtype=np.int64)))

    max_rows = max(len(s) for _, s in assignments)
    R = max(256, -(-max_rows // 256) * 256)

    mask = np.tril(np.ones((D, D), dtype=np.float32), -1)
    maskc = np.concatenate([mask, mask], axis=1)

    in_maps = []
    for ke, rows in assignments:
        yTc = np.zeros((D, R), dtype=np.float32)
        if len(rows):
            yTc[:, : len(rows)] = y[rows].T
        in_maps.append(
            {
                "yT": yTc,
                "Wm": _pack_weight(W[ke] * maskc),
                "bT": np.ascontiguousarray(b[ke].reshape(8, 128).T),
            }
        )

    nc = _get_program(R)

    if os.environ.get("BASS_KERNEL_SIM") == "1":
        results = _run_sim(nc, in_maps)
    else:
        results = _run_hw(nc, in_maps)
    LAST_RESULTS = results

    for (ke, rows), res in zip(assignments, results):
        if len(rows):
            # device returns y*exp(ls + b_ls) + shift; add b_shift here
            out[rows] = res["uT"][:, : len(rows)].T + b[ke, :D][None, :]
    return out


def _install_ntff_hook_shim():
    """This image's antenv lacks axon_hooks; wire the ctypes NTFF hook up
    ourselves so run_bass_kernel_spmd(trace=True) can profile."""
    import sys
    import types

    if "antenv.axon_hooks" in sys.modules:
        return
    try:
        from trn_agent_boot.trn_boot import _ntff_profile_via_ctypes

        hook = _ntff_profile_via_ctypes("/opt/axon/libaxon_pjrt.so")
    except Exception:
        hook = None
    mod = types.ModuleType("antenv.axon_hooks")
    mod.get_axon_ntff_profile_hook = lambda: hook
    sys.modules["antenv.axon_hooks"] = mod


def _run_hw(nc, in_maps):
    global LAST_EXEC_NS
    _ensure_path()
    from concourse import bass_utils

    trace = os.environ.get("BASS_KERNEL_TRACE") == "1"
    if trace:
        _install_ntff_hook_shim()
    res = bass_utils.run_bass_kernel_spmd(
        nc,
        in_maps,
        core_ids=list(range(len(in_maps))),
        trace=trace,
    )
    LAST_EXEC_NS = res.exec_time_ns
    return res.results


def _run_sim(nc, in_maps):
    """CoreSim the program (core 0's data on every core) — debug only."""
    _ensure_path()
    from concourse.bass_interp import CoreSim

    results = []
    for im in in_maps:
        sim = CoreSim(nc, trace=False, require_finite=True, require_nnan=True)
        for name, val in im.items():
            sim.tensor(name)[:] = val
        sim.simulate(check_with_hw=False)
        results.append({"uT": sim.tensor("uT").copy()})
    return results


# revision 46
# speedup vs baseline: 1.0688x; 1.0256x over previous
"""Trainium2 Bass kernel for nn_DiscreteSelectorTransform (MoE-style routed MAF).

Math (reference):
    h   = einsum('bd,kde->bke', y, W*mask) + b      # all K experts, dense
    sel = h[arange(B), x]                           # pick expert by label
    shift, log_scale = split(sel, 2, -1)
    u   = y * exp(log_scale) + shift
    out = where(x == 0, y, u)                       # expert 0 == identity

Instead of the dense all-experts compute (8x waste), we route on the host:
 - rows with label 0 are copied through on the host (identity expert),
 - remaining rows are grouped by expert and distributed over the 8 cores,
   one expert per core (big experts split across spare cores), so each
   core loads exactly one masked weight matrix (2MB) and runs a plain
   [rows, 512] @ [512, 1024] matmul + exp/mul/add epilogue for its rows.

Device layout: transposed orientation.  Each core receives y^T [512, R]
(features on partitions), computes h^T = Wm^T @ y^T into PSUM as 8
[128, N] e-chunks (W d-chunk stationary, y^T moving, float32r full-rate
mode), then the epilogue
    u^T = y^T * exp(ls^T + b_ls) + (sh^T + b_sh)
runs on ACT (exp / bias-add, per-partition bias) + DVE (mul / add) in the
same transposed layout and streams u^T [512, R] back out.  The host
inverts the permutation.
"""

import os
import numpy as np

_TRN_REPO = "/opt/trn_rl_repo"


def _ensure_path():
    try:
        import concourse  # noqa: F401
    except ImportError:
        import sys

        sys.path.insert(0, _TRN_REPO)


_PROGRAM_CACHE = {}

LAST_EXEC_NS = None
LAST_RESULTS = None


def _build_program(R):
    """One SPMD program: R rows (multiple of 256), one expert weight per core.

    Rows are processed in groups of 512 (plus one 256 tail when R % 512):
    512-wide DMA lines are at the HWDGE efficiency knee, and both widths
    keep float32r matmuls at full rate (moving dim >= 256).

    The weight arrives column-PACKED per d-chunk: chunk c keeps only its
    2*(c+1) nonzero e-blocks [shift j<=c | log_scale j<=c], so the DMA
    moves 20/32 of the dense masked weight.
    """
    _ensure_path()
    import concourse.mybir as mybir
    import concourse.tile as tile
    from concourse import bacc
    from concourse.vector_clock import ScopedClock

    class _LeanTileContext(tile.TileContext):
        """Tile's stock exit costs ~12us on HW: drain + two all-engine EVSEM
        butterflies around the gpsimd sem cleanup.  The second butterfly only
        protects against a next execution racing the cleanup, but the
        bass/runtime end-of-module barrier (which every engine executes right
        after this block) already provides that rendezvous — so a sem-only
        barrier plus cleanup suffices.  (A leaner drain->gpsimd semaphore
        handoff instead of the barrier measured the same within noise.)"""

        def _drain_and_barrier(self, tick_clock, wait_clock):
            drain_inst = self.nc.sync.drain()
            wait_clock.add_sem_waits(
                drain_inst.ins, ScopedClock({None: tick_clock.global_clock})
            )
            self.nc.all_engine_barrier(sem_only=True)
            popped = self.nc._tile_sem_poison_stack.pop()
            assert popped is self._sem_poison
            self.nc.clear_and_free_semaphores(list(self.sems.allocated().values()))

    assert R % 256 == 0
    widths = [512] * (R // 512) + ([256] if R % 512 else [])
    D = 512
    E = 1024

    nc = bacc.Bacc(
        "TRN2",
        target_bir_lowering=False,
        debug=False,
        enable_asserts=False,
        num_devices=8,
        enable_partition_id=False,
    )
    f32 = mybir.dt.float32
    f32r = mybir.dt.float32r
    Exp = mybir.ActivationFunctionType.Exp

    # y / W live as float32r end-to-end: same fp32 bits host-side, but the
    # BIR verifier requires f32r matmult operands to be produced as f32r.
    yT = nc.dram_tensor("yT", [D, R], f32r, kind="ExternalInput").ap()
    Wm = nc.dram_tensor("Wm", [D, E], f32r, kind="ExternalInput").ap()
    bT = nc.dram_tensor("bT", [128, 8], f32, kind="ExternalInput").ap()
    uT = nc.dram_tensor("uT", [D, R], f32, kind="ExternalOutput").ap()

    with _LeanTileContext(nc) as tc:
        with (
            tc.tile_pool(name="wp", bufs=1) as wp,
            tc.tile_pool(name="yp", bufs=6) as yp,
            tc.tile_pool(name="up", bufs=3) as up,
            tc.tile_pool(name="ep", bufs=2) as ep,
            tc.tile_pool(name="pp", bufs=1, space="PSUM") as pp,
        ):
            # One-time loads: packed masked weight (4 d-chunks) + bias.
            # The first matmul needs W chunk 0 + y chunk 0: put W chunk 0
            # first on the SP queue (ahead of y), the rest on the ACT HWDGE
            # (whose queue opens later, after its activation-table load).
            wts = []
            for c in range(4):
                ncols = 256 * (c + 1)  # 2*(c+1) nonzero e-blocks
                wt = wp.tile([128, ncols], f32r, name=f"w{c}", tag=f"w{c}")
                eng = nc.sync if c == 0 else nc.scalar
                eng.dma_start(out=wt, in_=Wm[128 * c : 128 * (c + 1), :ncols])
                wts.append(wt)
            bt = wp.tile([128, 8], f32, name="bt", tag="bt")
            nc.scalar.dma_start(out=bt, in_=bT)

            # PE warm-up: HAM un-throttles the PE clock (1.2 -> 2.4 GHz) only
            # after ~3.4us of sustained activity, and the first real matmul
            # can't start before ~13us (fill DMA latency) — so the first ~30
            # real matmuls measured cold.  The PE is idle from ~7.5us while
            # the fill streams in; fill that window with throwaway matmuls on
            # a zeroed scratch tile, sized to end right as the data lands.
            # They share the psh0 PSUM slot, so group 0 naturally serializes
            # after them, and start=True on the real matmul discards them.
            dummy = wp.tile([128, 640], f32r, name="dummy", tag="dummy")
            nc.vector.memzero(dummy)
            pdum = pp.tile([128, 512], f32, name="pdum", tag="psh0")
            for _ in range(12):
                nc.tensor.matmul(
                    pdum, dummy[:, :128], dummy[:, 128:640], start=True, stop=True
                )

            off = 0
            for g, N in enumerate(widths):
                sl = slice(off, off + N)
                off += N
                # One DMA per group: d-chunk c lands at columns [c*N, (c+1)*N).
                # The pipeline-fill groups are split per chunk instead, so the
                # first matmul only waits for 256KB, not the whole group.
                yc = yp.tile([128, 4 * N], f32r, name="yc", tag="yc")
                if g < 2:
                    for c in range(4):
                        nc.sync.dma_start(
                            out=yc[:, c * N : (c + 1) * N],
                            in_=yT[128 * c : 128 * (c + 1), sl],
                        )
                else:
                    nc.sync.dma_start(
                        out=yc, in_=yT[:, sl].rearrange("(c q) n -> q c n", c=4)
                    )
                yts = [yc[:, c * N : (c + 1) * N] for c in range(4)]

                # e-chunk j of h^T accumulates over d-chunks c >= j only:
                # the AR mask m[d, i] = (i < d) zeroes every (c, j) weight
                # block with c < j, in both the shift and log_scale halves.
                # In chunk c's packed weight, shift block j sits at column
                # 128*j and log_scale block j at 128*(c+1+j).
                # Feature chunks are processed in PAIRS (2 PSUM banks per
                # tile) so each epilogue op covers 2*N elements — halves the
                # op count and, more importantly, the semaphore traffic.
                # b_shift is added host-side during unshard.
                for p in range(2):
                    psh = pp.tile([128, 2 * N], f32, name=f"psh{p}", tag=f"psh{p}")
                    pls = pp.tile([128, 2 * N], f32, name=f"pls{p}", tag=f"pls{p}")
                    et = ep.tile([128, 2 * N], f32, name=f"et{p}", tag=f"e{p}")
                    for h in range(2):
                        j = 2 * p + h
                        hs = slice(h * N, (h + 1) * N)
                        for c in range(j, 4):
                            nc.tensor.matmul(
                                psh[:, hs],
                                wts[c][:, 128 * j : 128 * (j + 1)],
                                yts[c],
                                start=(c == j),
                                stop=(c == 3),
                            )
                        for c in range(j, 4):
                            blk = c + 1 + j
                            nc.tensor.matmul(
                                pls[:, hs],
                                wts[c][:, 128 * blk : 128 * (blk + 1)],
                                yts[c],
                                start=(c == j),
                                stop=(c == 3),
                            )
                        nc.scalar.activation(
                            et[:, hs], pls[:, hs], Exp, bias=bt[:, 4 + j : 5 + j]
                        )
                    ut = up.tile([128, 2 * N], f32, name=f"ut{p}", tag=f"u{p}")
                    nc.vector.tensor_mul(
                        ut, yc[:, 2 * p * N : (2 * p + 2) * N].bitcast(f32), et
                    )
                    nc.vector.tensor_add(ut, ut, psh)
                    # alternate output DMAs between the two HWDGE queues
                    dma_eng = nc.sync if p == 0 else nc.scalar
                    dma_eng.dma_start(
                        out=uT[256 * p : 256 * (p + 1), sl].rearrange(
                            "(a q) n -> q a n", a=2
                        ),
                        in_=ut,
                    )

    nc.compile()
    return nc


def _get_program(R):
    if R not in _PROGRAM_CACHE:
        _PROGRAM_CACHE[R] = _build_program(R)
    return _PROGRAM_CACHE[R]


def _pack_weight(Wmk):
    """Column-pack the masked weight: d-chunk c keeps its 2*(c+1) nonzero
    e-blocks as [shift j=0..c | log_scale j=0..c]."""
    Wp = np.zeros((512, 1024), dtype=np.float32)
    for c in range(4):
        rows = slice(128 * c, 128 * (c + 1))
        for j in range(c + 1):
            Wp[rows, 128 * j : 128 * (j + 1)] = Wmk[rows, 128 * j : 128 * (j + 1)]
            Wp[rows, 128 * (c + 1 + j) : 128 * (c + 2 + j)] = Wmk[
                rows, 512 + 128 * j : 512 + 128 * (j + 1)
            ]
    return Wp


def _route(counts, n_cores):
    """Cores per expert: start 1 each (nonzero), grow the most loaded."""
    m = np.where(counts > 0, 1, 0).astype(np.int64)
    if m.sum() == 0:
        return m
    while m.sum() < n_cores:
        load = np.where(m > 0, counts / np.maximum(m, 1), -1.0)
        m[int(np.argmax(load))] += 1
    return m


def kernel(y, W, b, x):
    global LAST_EXEC_NS, LAST_RESULTS
    y = np.ascontiguousarray(np.asarray(y, dtype=np.float32))
    W = np.asarray(W, dtype=np.float32)
    b = np.asarray(b, dtype=np.float32)
    xl = np.asarray(x).astype(np.int64).ravel()

    B, D = y.shape
    K, _, E = W.shape
    n_cores = 8

    out = np.empty((B, D), dtype=np.float32)
    id_rows = xl == 0
    out[id_rows] = y[id_rows]

    rows_per_expert = [np.nonzero(xl == k)[0] for k in range(1, K)]
    counts = np.array([len(r) for r in rows_per_expert], dtype=np.int64)
    if counts.sum() == 0:
        return out

    m = _route(counts, n_cores)
    assignments = []  # (expert index 1..K-1, row ids)
    for ki, rows in enumerate(rows_per_expert):
        if m[ki] == 0:
            continue
        for s in np.array_split(rows, m[ki]):
            assignments.append((ki + 1, s))
    while len(assignments) < n_cores:
        assignments.append((1, np.array([], dtype=np.int64)))

    max_rows = max(len(s) for _, s in assignments)
    R = max(256, -(-max_rows // 256) * 256)

    mask = np.tril(np.ones((D, D), dtype=np.float32), -1)
    maskc = np.concatenate([mask, mask], axis=1)

    in_maps = []
    for ke, rows in assignments:
        yTc = np.zeros((D, R), dtype=np.float32)
        if len(rows):
            yTc[:, : len(rows)] = y[rows].T
        in_maps.append(
            {
                "yT": yTc,
                "Wm": _pack_weight(W[ke] * maskc),
                "bT": np.ascontiguousarray(b[ke].reshape(8, 128).T),
            }
        )

    nc = _get_program(R)

    if os.environ.get("BASS_KERNEL_SIM") == "1":
        results = _run_sim(nc, in_maps)
    else:
        results = _run_hw(nc, in_maps)
    LAST_RESULTS = results

    for (ke, rows), res in zip(assignments, results):
        if len(rows):
            # device returns y*exp(ls + b_ls) + shift; add b_shift here
            out[rows] = res["uT"][:, : len(rows)].T + b[ke, :D][None, :]
    return out


def _install_ntff_hook_shim():
    """This image's antenv lacks axon_hooks; wire the ctypes NTFF hook up
    ourselves so run_bass_kernel_spmd(trace=True) can profile."""
    import sys
    import types

    if "antenv.axon_hooks" in sys.modules:
        return
    try:
        from trn_agent_boot.trn_boot import _ntff_profile_via_ctypes

        hook = _ntff_profile_via_ctypes("/opt/axon/libaxon_pjrt.so")
    except Exception:
        hook = None
    mod = types.ModuleType("antenv.axon_hooks")
    mod.get_axon_ntff_profile_hook = lambda: hook
    sys.modules["antenv.axon_hooks"] = mod


def _run_hw(nc, in_maps):
    global LAST_EXEC_NS
    _ensure_path()
    from concourse import bass_utils

    trace = os.environ.get("BASS_KERNEL_TRACE") == "1"
    if trace:
        _install_ntff_hook_shim()
    res = bass_utils.run_bass_kernel_spmd(
        nc,
        in_maps,
        core_ids=list(range(len(in_maps))),
        trace=trace,
    )
    LAST_EXEC_NS = res.exec_time_ns
    return res.results


def _run_sim(nc, in_maps):
    """CoreSim the program (core 0's data on every core) — debug only."""
    _ensure_path()
    from concourse.bass_interp import CoreSim

    results = []
    for im in in_maps:
        sim = CoreSim(nc, trace=False, require_finite=True, require_nnan=True)
        for name, val in im.items():
            sim.tensor(name)[:] = val
        sim.simulate(check_with_hw=False)
        results.append({"uT": sim.tensor("uT").copy()})
    return results
